# revision 7
# baseline (speedup 1.0000x reference)
"""RBF Gram matrix kernel for TRN2: out[i,j] = exp(-||x_i - y_j||^2).

x, y: [8192, 64] fp32 -> out [8192, 8192] fp32.

Sharding: x rows split across 8 NeuronCores (1024 rows each), y
replicated. Each core computes a [1024, 8192] tile of the Gram matrix.

Math: one K=128 matmul pass per output tile, mixed precision at the
full bf16 moving rate (fp16 moving runs at half rate, bf16-only needs
two passes for the required mantissa):
  stationary (fp16): [x16(64); 1; 1; x16[0:62]]
  moving     (bf16): [2*bf16(y)(64); -ysq_h; -ysq_l; 2*bf16(y-yh)[0:62]]
PSUM accumulates 2*x.y - |y|^2 in fp32; the -|x_i|^2 term rides the
ScalarE activation bias (fp32 per-partition AP), so one ACT pass
computes exp(PSUM + bias) = exp(-s) and writes bf16 to SBUF; each
128-row tile leaves as one contiguous 2 MiB DMA (the final tile is
split per 512 KiB block to shorten the end-of-kernel drain tail).

The output leaves the device as bf16 (the host widens to fp32): s is in
~[39, 250] here, so bf16's 8-bit mantissa costs at most 2^-9 relative
to the output absmax. The fp16/bf16 input rounding perturbs s by
sigma ~5e-3; the rel-err-vs-absmax metric only exposes entries within
~0.5 of the dataset's s_min, where exp(-(s-s_min)) ~ 1. Measured
end-to-end rel err on the seed-0 dataset: ~2.6e-3 (budget 2e-2).

Per core per pass the engines carry: PE 128 N=512 matmuls (~30 us),
ScalarE 32 exp tiles (~63 us <- the roofline), DMA 19 MiB (~53 us).
"""

import numpy as np
import ml_dtypes

import concourse.bass as bass
import concourse.tile as tile
import concourse.mybir as mybir
from concourse.bass_utils import run_bass_kernel_spmd

N_CORES = 8
N_ROWS = 8192          # x rows (Gram rows), sharded
N_COLS = 8192          # y rows (Gram cols), replicated
D = 64
RPC = N_ROWS // N_CORES  # 1024 rows per core

DT = mybir.dt.float32
BF = mybir.dt.bfloat16
F16 = mybir.dt.float16
NL = 62                # y-lo correction dims carried in rows 66..127

R_TILES = RPC // 128   # 8 row tiles of 128 partitions
CBW = 2048             # column block width (4 PSUM banks)
MM_W = 512             # one matmul free dim (1 PSUM bank, fp32)
C_BLOCKS = N_COLS // CBW

# legacy aliases (experiment harnesses import these)
K1 = 2 * D
K2 = D + 4
W = RPC + N_COLS


def _split_excess_waits(nc, limits=None):
    """The walrus in this container accepts only a small number of sync-wait
    commands per instruction (1 for Drain, ~2 elsewhere). Hoist excess waits
    onto injected NoOps on the same engine, placed just before the original
    instruction so per-engine ordering (and thus the waits) is preserved."""
    if limits is None:
        limits = {"InstNoOp": 1, "default": 1}
    n_split = 0
    for f in nc.m.functions:
        for blk in f.blocks:
            insts = blk.instructions
            i = 0
            while i < len(insts):
                inst = insts[i]
                si = inst.sync_info
                lim = limits.get(type(inst).__name__, limits["default"])
                if si is not None and len(si.on_wait) > lim:
                    waits = list(si.on_wait)
                    keep = waits[-lim:] if lim > 0 else []
                    excess = waits[:-lim] if lim > 0 else waits
                    per_nop = limits["InstNoOp"]
                    chunks = [
                        excess[j:j + per_nop] for j in range(0, len(excess), per_nop)
                    ]
                    for k, ch in enumerate(chunks):
                        nop = mybir.InstNoOp(
                            name=nc.get_next_instruction_name(),
                            sync_info=mybir.SyncInfo(on_wait=ch, on_update=[]),
                            bass_nofuse=True,
                            engine=inst.engine,
                        )
                        nc.register_instruction(nop)
                        insts.insert(i + k, nop)
                    si.on_wait = keep
                    i += len(chunks)
                    n_split += 1
                i += 1
    return n_split


def _dedup_ldweights(nc):
    """Walrus runs with --enable-ldw-opt=false, so every InstMatmult gets
    its own InstLdweights even when consecutive matmuls share the same
    stationary operand -- each reload costs ~70-110 ns of serial PE time.
    Delete an InstLdweights whose weight AP is identical to the previous
    one on the PE stream (nothing else mutates the PE array), merging its
    waits into the next PE instruction. LDWs carry no semaphore updates,
    and waits are >=-monotonic, so the merge preserves synchronization."""
    n = 0
    for f in nc.m.functions:
        for blk in f.blocks:
            insts = blk.instructions
            last_sig = None
            carry = []
            i = 0
            while i < len(insts):
                inst = insts[i]
                t = type(inst).__name__
                if str(inst.engine) != "EngineType.PE":
                    i += 1
                    continue
                if carry:
                    si = inst.sync_info
                    if si is None:
                        inst.sync_info = mybir.SyncInfo(
                            on_wait=carry, on_update=[])
                    else:
                        si.on_wait = list(si.on_wait) + carry
                    carry = []
                if t == "InstLdweights":
                    sig = str(inst.ins[0])
                    si = inst.sync_info
                    assert si is None or not si.on_update
                    if sig == last_sig:
                        carry = list(si.on_wait) if si else []
                        del insts[i]
                        n += 1
                        continue
                    last_sig = sig
                elif t == "InstMatmult":
                    pass
                else:
                    last_sig = None  # unknown PE inst: be conservative
                i += 1
            assert not carry
    return n


def finalize_nc(nc):
    _dedup_ldweights(nc)
    _split_excess_waits(nc)


def emit_loads(nc, sbin, q_d, m_d, xb_d):
    """Input DMAs (all on the SP HWDGE ring)."""
    q_t = sbin.tile([128, RPC], F16, name="q")
    xb_t = sbin.tile([128, R_TILES], DT, name="xb")
    m_t = [sbin.tile([128, CBW], BF, name=f"m{h}") for h in range(C_BLOCKS)]
    nc.sync.dma_start(q_t[:], q_d[:, :])
    nc.sync.dma_start(xb_t[:], xb_d[:, :])
    for h in range(C_BLOCKS):
        nc.sync.dma_start(m_t[h][:], m_d[:, h * CBW:(h + 1) * CBW])
    return q_t, xb_t, m_t


def emit_body(nc, sbin, sbout, ps, q_d, m_d, xb_d, out_d,
              mm=True, act=True, store=True, tailsplit=True,
              tiles=None, endload=False):
    """One full pass: matmuls, biased Exp, output DMAs (and input DMAs:
    at the top by default, or at the bottom as a next-iteration prefetch
    when `tiles` come from a preamble emit_loads and endload=True).
    The mm/act/store flags carve out stages for bottleneck ablation.
    ScalarE runs only ACTIVATEs; all DMA issue rides the SP ring."""
    if tiles is None:
        tiles = emit_loads(nc, sbin, q_d, m_d, xb_d)
    q_t, xb_t, m_t = tiles

    if endload == "front":
        # re-issue the full input DMAs at the top of the body: they fill
        # the second buffer of the double-buffered input pool (compute
        # reads the preamble copy), and their SP-ring descriptors run
        # AHEAD of this pass's stores instead of serializing after them
        # at the pass tail
        emit_loads(nc, sbin, q_d, m_d, xb_d)

    for r in range(R_TILES):
        lhs = q_t[:, r * 128:(r + 1) * 128]
        bias = xb_t[:, r:r + 1]
        ot = sbout.tile([128, N_COLS], BF, name="ot")
        split = tailsplit and (tailsplit == "all" or r == R_TILES - 1)
        sw = CBW // 2 if tailsplit == "fine" and r == R_TILES - 1 else CBW
        for cb in range(C_BLOCKS):
            acc = ps.tile([128, CBW], DT)
            if mm:
                for j in range(CBW // MM_W):
                    seg = slice(j * MM_W, (j + 1) * MM_W)
                    nc.tensor.matmul(
                        acc[:, seg], lhs, m_t[cb][:, seg],
                        start=True, stop=True,
                    )
            if act:
                nc.scalar.activation(
                    ot[:, cb * CBW:(cb + 1) * CBW], acc[:],
                    mybir.ActivationFunctionType.Exp, bias=bias,
                )
                if store and split:
                    for c0 in range(cb * CBW, (cb + 1) * CBW, sw):
                        nc.sync.dma_start(
                            out_d[r * 128:(r + 1) * 128, c0:c0 + sw],
                            ot[:, c0:c0 + sw],
                        )
        if act and store and not split:
            nc.sync.dma_start(out_d[r * 128:(r + 1) * 128, :], ot[:])
    if endload is True:
        emit_loads(nc, sbin, q_d, m_d, xb_d)


def build_nc():
    nc = bass.Bass()
    q_d = nc.dram_tensor("q", [128, RPC], F16, kind="ExternalInput")
    m_d = nc.dram_tensor("m", [128, N_COLS], BF, kind="ExternalInput")
    xb_d = nc.dram_tensor("xb", [128, R_TILES], DT, kind="ExternalInput")
    out_d = nc.dram_tensor("out", [RPC, N_COLS], BF, kind="ExternalOutput")

    with tile.TileContext(nc) as tc:
        with (
            tc.tile_pool(name="inp", bufs=1) as sbin,
            tc.tile_pool(name="outp", bufs=4) as sbout,
            tc.tile_pool(name="ps", bufs=2, space="PSUM") as ps,
        ):
            # warm the ACT exp table-set load (~2.7 us) under the input DMAs
            warm = sbout.tile([128, 8], DT, name="actwarm")
            nc.scalar.activation(warm[:], warm[:], mybir.ActivationFunctionType.Exp)
            emit_body(nc, sbin, sbout, ps, q_d, m_d, xb_d, out_d)
    finalize_nc(nc)
    return nc


def prepare_inputs(x, y):
    """Host-side prep: fp16/bf16 encodes, transposes, per-core maps."""
    x = np.asarray(x, dtype=np.float32)
    y = np.asarray(y, dtype=np.float32)
    assert x.shape == (N_ROWS, D) and y.shape == (N_COLS, D)

    x_sq = (x * x).sum(axis=1, dtype=np.float32)
    y_sq = (y * y).sum(axis=1, dtype=np.float32)

    x16 = x.astype(np.float16)
    yh = y.astype(ml_dtypes.bfloat16)
    y2h = (2.0 * yh.astype(np.float32)).astype(ml_dtypes.bfloat16)
    yl2 = (2.0 * (y - yh.astype(np.float32))).astype(ml_dtypes.bfloat16)
    ysq_h = y_sq.astype(ml_dtypes.bfloat16)
    ysq_l = (y_sq - ysq_h.astype(np.float32)).astype(ml_dtypes.bfloat16)

    # moving map, shared by all cores: [128, N_COLS] bf16
    m = np.zeros((128, N_COLS), ml_dtypes.bfloat16)
    m[:D] = y2h.T
    m[D] = -ysq_h
    m[D + 1] = -ysq_l
    m[D + 2:D + 2 + NL] = yl2.T[:NL]

    in_maps = []
    for c in range(N_CORES):
        rows = slice(c * RPC, (c + 1) * RPC)
        q = np.zeros((128, RPC), np.float16)
        q[:D] = x16.T[:, rows]
        q[D] = 1.0
        q[D + 1] = 1.0
        q[D + 2:D + 2 + NL] = x16.T[:NL, rows]
        xb = (-x_sq[rows]).astype(np.float32).reshape(R_TILES, 128).T.copy()
        in_maps.append({"q": q, "m": m, "xb": xb})
    return in_maps


def kernel(x, y):
    in_maps = prepare_inputs(x, y)
    nc = build_nc()
    res = run_bass_kernel_spmd(nc, in_maps, core_ids=list(range(N_CORES)))
    out = np.concatenate([res.results[c]["out"] for c in range(N_CORES)], axis=0)
    return out.astype(np.float32)


# revision 8
# speedup vs baseline: 1.0268x; 1.0268x over previous
"""RBF Gram matrix kernel for TRN2: out[i,j] = exp(-||x_i - y_j||^2).

x, y: [8192, 64] fp32 -> out [8192, 8192] fp32.

Sharding: x rows split across 8 NeuronCores (1024 rows each), y
replicated. Each core computes a [1024, 8192] tile of the Gram matrix.

Math: one K=128 matmul pass per output tile, mixed precision at the
full bf16 moving rate (fp16 moving runs at half rate, bf16-only needs
two passes for the required mantissa):
  stationary (fp16): [x16(64); 1; 1; x16[0:62]]
  moving     (bf16): [2*bf16(y)(64); -ysq_h; -ysq_l; 2*bf16(y-yh)[0:62]]
PSUM accumulates 2*x.y - |y|^2 in fp32; the -|x_i|^2 term rides the
ScalarE activation bias (fp32 per-partition AP), so one ACT pass
computes exp(PSUM + bias) = exp(-s) and writes bf16 to SBUF; stores
leave per 512 KiB column block as each Exp tile completes, which
spreads the SP-ring store traffic and keeps the end-of-kernel DMA
drain tail to one block.

The output leaves the device as bf16 (the host widens to fp32): s is in
~[39, 250] here, so bf16's 8-bit mantissa costs at most 2^-9 relative
to the output absmax. The fp16/bf16 input rounding perturbs s by
sigma ~5e-3; the rel-err-vs-absmax metric only exposes entries within
~0.5 of the dataset's s_min, where exp(-(s-s_min)) ~ 1. Measured
end-to-end rel err on the seed-0 dataset: ~2.6e-3 (budget 2e-2).

Per core per pass the engines carry: PE 128 N=512 matmuls (~30 us),
ScalarE 32 exp tiles (~63 us <- the roofline), DMA 19 MiB (~53 us).
"""

import numpy as np
import ml_dtypes

import concourse.bass as bass
import concourse.tile as tile
import concourse.mybir as mybir
from concourse.bass_utils import run_bass_kernel_spmd

N_CORES = 8
N_ROWS = 8192          # x rows (Gram rows), sharded
N_COLS = 8192          # y rows (Gram cols), replicated
D = 64
RPC = N_ROWS // N_CORES  # 1024 rows per core

DT = mybir.dt.float32
BF = mybir.dt.bfloat16
F16 = mybir.dt.float16
NL = 62                # y-lo correction dims carried in rows 66..127

R_TILES = RPC // 128   # 8 row tiles of 128 partitions
CBW = 2048             # column block width (4 PSUM banks)
MM_W = 512             # one matmul free dim (1 PSUM bank, fp32)
C_BLOCKS = N_COLS // CBW

# legacy aliases (experiment harnesses import these)
K1 = 2 * D
K2 = D + 4
W = RPC + N_COLS


def _split_excess_waits(nc, limits=None):
    """The walrus in this container accepts only a small number of sync-wait
    commands per instruction (1 for Drain, ~2 elsewhere). Hoist excess waits
    onto injected NoOps on the same engine, placed just before the original
    instruction so per-engine ordering (and thus the waits) is preserved."""
    if limits is None:
        limits = {"InstNoOp": 1, "default": 1}
    n_split = 0
    for f in nc.m.functions:
        for blk in f.blocks:
            insts = blk.instructions
            i = 0
            while i < len(insts):
                inst = insts[i]
                si = inst.sync_info
                lim = limits.get(type(inst).__name__, limits["default"])
                if si is not None and len(si.on_wait) > lim:
                    waits = list(si.on_wait)
                    keep = waits[-lim:] if lim > 0 else []
                    excess = waits[:-lim] if lim > 0 else waits
                    per_nop = limits["InstNoOp"]
                    chunks = [
                        excess[j:j + per_nop] for j in range(0, len(excess), per_nop)
                    ]
                    for k, ch in enumerate(chunks):
                        nop = mybir.InstNoOp(
                            name=nc.get_next_instruction_name(),
                            sync_info=mybir.SyncInfo(on_wait=ch, on_update=[]),
                            bass_nofuse=True,
                            engine=inst.engine,
                        )
                        nc.register_instruction(nop)
                        insts.insert(i + k, nop)
                    si.on_wait = keep
                    i += len(chunks)
                    n_split += 1
                i += 1
    return n_split


def _dedup_ldweights(nc):
    """Walrus runs with --enable-ldw-opt=false, so every InstMatmult gets
    its own InstLdweights even when consecutive matmuls share the same
    stationary operand -- each reload costs ~70-110 ns of serial PE time.
    Delete an InstLdweights whose weight AP is identical to the previous
    one on the PE stream (nothing else mutates the PE array), merging its
    waits into the next PE instruction. LDWs carry no semaphore updates,
    and waits are >=-monotonic, so the merge preserves synchronization."""
    n = 0
    for f in nc.m.functions:
        for blk in f.blocks:
            insts = blk.instructions
            last_sig = None
            carry = []
            i = 0
            while i < len(insts):
                inst = insts[i]
                t = type(inst).__name__
                if str(inst.engine) != "EngineType.PE":
                    i += 1
                    continue
                if carry:
                    si = inst.sync_info
                    if si is None:
                        inst.sync_info = mybir.SyncInfo(
                            on_wait=carry, on_update=[])
                    else:
                        si.on_wait = list(si.on_wait) + carry
                    carry = []
                if t == "InstLdweights":
                    sig = str(inst.ins[0])
                    si = inst.sync_info
                    assert si is None or not si.on_update
                    if sig == last_sig:
                        carry = list(si.on_wait) if si else []
                        del insts[i]
                        n += 1
                        continue
                    last_sig = sig
                elif t == "InstMatmult":
                    pass
                else:
                    last_sig = None  # unknown PE inst: be conservative
                i += 1
            assert not carry
    return n


def finalize_nc(nc):
    _dedup_ldweights(nc)
    _split_excess_waits(nc)


def emit_loads(nc, sbin, q_d, m_d, xb_d):
    """Input DMAs (all on the SP HWDGE ring)."""
    q_t = sbin.tile([128, RPC], F16, name="q")
    xb_t = sbin.tile([128, R_TILES], DT, name="xb")
    m_t = [sbin.tile([128, CBW], BF, name=f"m{h}") for h in range(C_BLOCKS)]
    nc.sync.dma_start(q_t[:], q_d[:, :])
    nc.sync.dma_start(xb_t[:], xb_d[:, :])
    for h in range(C_BLOCKS):
        nc.sync.dma_start(m_t[h][:], m_d[:, h * CBW:(h + 1) * CBW])
    return q_t, xb_t, m_t


def emit_body(nc, sbin, sbout, ps, q_d, m_d, xb_d, out_d,
              mm=True, act=True, store=True, tailsplit=True,
              tiles=None, endload=False):
    """One full pass: matmuls, biased Exp, output DMAs (and input DMAs:
    at the top by default, or at the bottom as a next-iteration prefetch
    when `tiles` come from a preamble emit_loads and endload=True).
    The mm/act/store flags carve out stages for bottleneck ablation.
    ScalarE runs only ACTIVATEs; all DMA issue rides the SP ring."""
    if tiles is None:
        tiles = emit_loads(nc, sbin, q_d, m_d, xb_d)
    q_t, xb_t, m_t = tiles

    if endload == "front":
        # re-issue the full input DMAs at the top of the body: they fill
        # the second buffer of the double-buffered input pool (compute
        # reads the preamble copy), and their SP-ring descriptors run
        # AHEAD of this pass's stores instead of serializing after them
        # at the pass tail
        emit_loads(nc, sbin, q_d, m_d, xb_d)

    for r in range(R_TILES):
        lhs = q_t[:, r * 128:(r + 1) * 128]
        bias = xb_t[:, r:r + 1]
        ot = sbout.tile([128, N_COLS], BF, name="ot")
        split = tailsplit and (tailsplit == "all" or r == R_TILES - 1)
        sw = CBW // 2 if tailsplit == "fine" and r == R_TILES - 1 else CBW
        for cb in range(C_BLOCKS):
            acc = ps.tile([128, CBW], DT)
            if mm:
                for j in range(CBW // MM_W):
                    seg = slice(j * MM_W, (j + 1) * MM_W)
                    nc.tensor.matmul(
                        acc[:, seg], lhs, m_t[cb][:, seg],
                        start=True, stop=True,
                    )
            if act:
                nc.scalar.activation(
                    ot[:, cb * CBW:(cb + 1) * CBW], acc[:],
                    mybir.ActivationFunctionType.Exp, bias=bias,
                )
                if store and split:
                    for c0 in range(cb * CBW, (cb + 1) * CBW, sw):
                        nc.sync.dma_start(
                            out_d[r * 128:(r + 1) * 128, c0:c0 + sw],
                            ot[:, c0:c0 + sw],
                        )
        if act and store and not split:
            nc.sync.dma_start(out_d[r * 128:(r + 1) * 128, :], ot[:])
    if endload is True:
        emit_loads(nc, sbin, q_d, m_d, xb_d)


def build_nc():
    nc = bass.Bass()
    q_d = nc.dram_tensor("q", [128, RPC], F16, kind="ExternalInput")
    m_d = nc.dram_tensor("m", [128, N_COLS], BF, kind="ExternalInput")
    xb_d = nc.dram_tensor("xb", [128, R_TILES], DT, kind="ExternalInput")
    out_d = nc.dram_tensor("out", [RPC, N_COLS], BF, kind="ExternalOutput")

    with tile.TileContext(nc) as tc:
        with (
            tc.tile_pool(name="inp", bufs=1) as sbin,
            tc.tile_pool(name="outp", bufs=4) as sbout,
            tc.tile_pool(name="ps", bufs=2, space="PSUM") as ps,
        ):
            # warm the ACT exp table-set load (~2.7 us) under the input DMAs
            warm = sbout.tile([128, 8], DT, name="actwarm")
            nc.scalar.activation(warm[:], warm[:], mybir.ActivationFunctionType.Exp)
            emit_body(nc, sbin, sbout, ps, q_d, m_d, xb_d, out_d,
                      tailsplit="all")
    finalize_nc(nc)
    return nc


def prepare_inputs(x, y):
    """Host-side prep: fp16/bf16 encodes, transposes, per-core maps."""
    x = np.asarray(x, dtype=np.float32)
    y = np.asarray(y, dtype=np.float32)
    assert x.shape == (N_ROWS, D) and y.shape == (N_COLS, D)

    x_sq = (x * x).sum(axis=1, dtype=np.float32)
    y_sq = (y * y).sum(axis=1, dtype=np.float32)

    x16 = x.astype(np.float16)
    yh = y.astype(ml_dtypes.bfloat16)
    y2h = (2.0 * yh.astype(np.float32)).astype(ml_dtypes.bfloat16)
    yl2 = (2.0 * (y - yh.astype(np.float32))).astype(ml_dtypes.bfloat16)
    ysq_h = y_sq.astype(ml_dtypes.bfloat16)
    ysq_l = (y_sq - ysq_h.astype(np.float32)).astype(ml_dtypes.bfloat16)

    # moving map, shared by all cores: [128, N_COLS] bf16
    m = np.zeros((128, N_COLS), ml_dtypes.bfloat16)
    m[:D] = y2h.T
    m[D] = -ysq_h
    m[D + 1] = -ysq_l
    m[D + 2:D + 2 + NL] = yl2.T[:NL]

    in_maps = []
    for c in range(N_CORES):
        rows = slice(c * RPC, (c + 1) * RPC)
        q = np.zeros((128, RPC), np.float16)
        q[:D] = x16.T[:, rows]
        q[D] = 1.0
        q[D + 1] = 1.0
        q[D + 2:D + 2 + NL] = x16.T[:NL, rows]
        xb = (-x_sq[rows]).astype(np.float32).reshape(R_TILES, 128).T.copy()
        in_maps.append({"q": q, "m": m, "xb": xb})
    return in_maps


def kernel(x, y):
    in_maps = prepare_inputs(x, y)
    nc = build_nc()
    res = run_bass_kernel_spmd(nc, in_maps, core_ids=list(range(N_CORES)))
    out = np.concatenate([res.results[c]["out"] for c in range(N_CORES)], axis=0)
    return out.astype(np.float32)


# revision 9
# speedup vs baseline: 1.0749x; 1.0468x over previous
"""RBF Gram matrix kernel for TRN2: out[i,j] = exp(-||x_i - y_j||^2).

x, y: [8192, 64] fp32 -> out [8192, 8192] fp32.

Sharding: x rows split across 8 NeuronCores (1024 rows each), y
replicated. Each core computes a [1024, 8192] tile of the Gram matrix.

Math: one K=128 matmul pass per output tile, mixed precision at the
full bf16 moving rate (fp16 moving runs at half rate, bf16-only needs
two passes for the required mantissa):
  stationary (fp16): [x16(64); 1; 1; x16[0:62]]
  moving     (bf16): [2*bf16(y)(64); -ysq_h; -ysq_l; 2*bf16(y-yh)[0:62]]
PSUM accumulates 2*x.y - |y|^2 in fp32; the -|x_i|^2 term rides the
ScalarE activation bias (fp32 per-partition AP), so one ACT pass
computes exp(PSUM + bias) = exp(-s) and writes bf16 to SBUF; stores
leave per 512 KiB column block as each Exp tile completes, which
spreads the SP-ring store traffic and keeps the end-of-kernel DMA
drain tail to one block.

The output leaves the device as bf16 (the host widens to fp32): s is in
~[39, 250] here, so bf16's 8-bit mantissa costs at most 2^-9 relative
to the output absmax. The fp16/bf16 input rounding perturbs s by
sigma ~5e-3; the rel-err-vs-absmax metric only exposes entries within
~0.5 of the dataset's s_min, where exp(-(s-s_min)) ~ 1. Measured
end-to-end rel err on the seed-0 dataset: ~2.6e-3 (budget 2e-2).

Per core per pass the engines carry: PE 128 N=512 matmuls (~30 us),
ScalarE 32 exp tiles (~63 us <- the roofline), DMA 19 MiB (~53 us).
"""

import numpy as np
import ml_dtypes

import concourse.bass as bass
import concourse.tile as tile
import concourse.mybir as mybir
from concourse.bass_utils import run_bass_kernel_spmd

N_CORES = 8
N_ROWS = 8192          # x rows (Gram rows), sharded
N_COLS = 8192          # y rows (Gram cols), replicated
D = 64
RPC = N_ROWS // N_CORES  # 1024 rows per core

DT = mybir.dt.float32
BF = mybir.dt.bfloat16
F16 = mybir.dt.float16
NL = 62                # y-lo correction dims carried in rows 66..127

R_TILES = RPC // 128   # 8 row tiles of 128 partitions
CBW = 2048             # column block width (4 PSUM banks)
MM_W = 512             # one matmul free dim (1 PSUM bank, fp32)
C_BLOCKS = N_COLS // CBW

# legacy aliases (experiment harnesses import these)
K1 = 2 * D
K2 = D + 4
W = RPC + N_COLS


def _split_excess_waits(nc, limits=None):
    """The walrus in this container accepts only a small number of sync-wait
    commands per instruction (1 for Drain, ~2 elsewhere). Hoist excess waits
    onto injected NoOps on the same engine, placed just before the original
    instruction so per-engine ordering (and thus the waits) is preserved."""
    if limits is None:
        limits = {"InstNoOp": 1, "default": 1}
    n_split = 0
    for f in nc.m.functions:
        for blk in f.blocks:
            insts = blk.instructions
            i = 0
            while i < len(insts):
                inst = insts[i]
                si = inst.sync_info
                lim = limits.get(type(inst).__name__, limits["default"])
                if si is not None and len(si.on_wait) > lim:
                    waits = list(si.on_wait)
                    keep = waits[-lim:] if lim > 0 else []
                    excess = waits[:-lim] if lim > 0 else waits
                    per_nop = limits["InstNoOp"]
                    chunks = [
                        excess[j:j + per_nop] for j in range(0, len(excess), per_nop)
                    ]
                    for k, ch in enumerate(chunks):
                        nop = mybir.InstNoOp(
                            name=nc.get_next_instruction_name(),
                            sync_info=mybir.SyncInfo(on_wait=ch, on_update=[]),
                            bass_nofuse=True,
                            engine=inst.engine,
                        )
                        nc.register_instruction(nop)
                        insts.insert(i + k, nop)
                    si.on_wait = keep
                    i += len(chunks)
                    n_split += 1
                i += 1
    return n_split


def _dedup_ldweights(nc):
    """Walrus runs with --enable-ldw-opt=false, so every InstMatmult gets
    its own InstLdweights even when consecutive matmuls share the same
    stationary operand -- each reload costs ~70-110 ns of serial PE time.
    Delete an InstLdweights whose weight AP is identical to the previous
    one on the PE stream (nothing else mutates the PE array), merging its
    waits into the next PE instruction. LDWs carry no semaphore updates,
    and waits are >=-monotonic, so the merge preserves synchronization."""
    n = 0
    for f in nc.m.functions:
        for blk in f.blocks:
            insts = blk.instructions
            last_sig = None
            carry = []
            i = 0
            while i < len(insts):
                inst = insts[i]
                t = type(inst).__name__
                if str(inst.engine) != "EngineType.PE":
                    i += 1
                    continue
                if carry:
                    si = inst.sync_info
                    if si is None:
                        inst.sync_info = mybir.SyncInfo(
                            on_wait=carry, on_update=[])
                    else:
                        si.on_wait = list(si.on_wait) + carry
                    carry = []
                if t == "InstLdweights":
                    sig = str(inst.ins[0])
                    si = inst.sync_info
                    assert si is None or not si.on_update
                    if sig == last_sig:
                        carry = list(si.on_wait) if si else []
                        del insts[i]
                        n += 1
                        continue
                    last_sig = sig
                elif t == "InstMatmult":
                    pass
                else:
                    last_sig = None  # unknown PE inst: be conservative
                i += 1
            assert not carry
    return n


def finalize_nc(nc):
    _dedup_ldweights(nc)
    _split_excess_waits(nc)


def emit_loads(nc, sbin, q_d, m_d, xb_d):
    """Input DMAs (all on the SP HWDGE ring)."""
    q_t = sbin.tile([128, RPC], F16, name="q")
    xb_t = sbin.tile([128, R_TILES], DT, name="xb")
    m_t = [sbin.tile([128, CBW], BF, name=f"m{h}") for h in range(C_BLOCKS)]
    nc.sync.dma_start(q_t[:], q_d[:, :])
    nc.sync.dma_start(xb_t[:], xb_d[:, :])
    for h in range(C_BLOCKS):
        nc.sync.dma_start(m_t[h][:], m_d[:, h * CBW:(h + 1) * CBW])
    return q_t, xb_t, m_t


def emit_body(nc, sbin, sbout, ps, q_d, m_d, xb_d, out_d,
              mm=True, act=True, store=True, tailsplit=True,
              tiles=None, endload=False, sbstage=None):
    """One full pass: matmuls, biased Exp, output DMAs (and input DMAs:
    at the top by default, or at the bottom as a next-iteration prefetch
    when `tiles` come from a preamble emit_loads and endload=True).
    The mm/act/store flags carve out stages for bottleneck ablation.
    ScalarE runs only ACTIVATEs; all DMA issue rides the SP ring."""
    if tiles is None:
        tiles = emit_loads(nc, sbin, q_d, m_d, xb_d)
    q_t, xb_t, m_t = tiles

    if endload == "front":
        # re-issue the full input DMAs at the top of the body: they fill
        # the second buffer of the double-buffered input pool (compute
        # reads the preamble copy), and their SP-ring descriptors run
        # AHEAD of this pass's stores instead of serializing after them
        # at the pass tail
        emit_loads(nc, sbin, q_d, m_d, xb_d)

    for r in range(R_TILES):
        lhs = q_t[:, r * 128:(r + 1) * 128]
        bias = xb_t[:, r:r + 1]
        ot = sbout.tile([128, N_COLS], BF, name="ot")
        split = tailsplit and (tailsplit == "all" or r == R_TILES - 1)
        sw = CBW // 2 if tailsplit == "fine" and r == R_TILES - 1 else CBW
        # with a staging pool, blocks 1..3 are evacuated PSUM->SBUF by the
        # (otherwise idle) Vector engine and batched into ONE SBUF-source
        # ACTIVATE (FD=6144), whose per-element cost is lower than three
        # PSUM-source ACTs -- the ScalarE stream drops ~0.9 us per r-tile
        stage = (sbstage.tile([128, (C_BLOCKS - 1) * CBW], DT, name="stg")
                 if sbstage is not None and act else None)
        for cb in range(C_BLOCKS):
            acc = ps.tile([128, CBW], DT)
            if mm:
                for j in range(CBW // MM_W):
                    seg = slice(j * MM_W, (j + 1) * MM_W)
                    nc.tensor.matmul(
                        acc[:, seg], lhs, m_t[cb][:, seg],
                        start=True, stop=True,
                    )
            if act:
                if stage is not None and cb > 0:
                    nc.vector.tensor_copy(
                        stage[:, (cb - 1) * CBW:cb * CBW], acc[:])
                else:
                    nc.scalar.activation(
                        ot[:, cb * CBW:(cb + 1) * CBW], acc[:],
                        mybir.ActivationFunctionType.Exp, bias=bias,
                    )
                    if store and split:
                        for c0 in range(cb * CBW, (cb + 1) * CBW, sw):
                            nc.sync.dma_start(
                                out_d[r * 128:(r + 1) * 128, c0:c0 + sw],
                                ot[:, c0:c0 + sw],
                            )
        if act and stage is not None:
            nc.scalar.activation(
                ot[:, CBW:], stage[:],
                mybir.ActivationFunctionType.Exp, bias=bias,
            )
            if store and split:
                for c0 in range(CBW, N_COLS, sw):
                    nc.sync.dma_start(
                        out_d[r * 128:(r + 1) * 128, c0:c0 + sw],
                        ot[:, c0:c0 + sw],
                    )
        if act and store and not split:
            nc.sync.dma_start(out_d[r * 128:(r + 1) * 128, :], ot[:])
    if endload is True:
        emit_loads(nc, sbin, q_d, m_d, xb_d)


def build_nc():
    nc = bass.Bass()
    q_d = nc.dram_tensor("q", [128, RPC], F16, kind="ExternalInput")
    m_d = nc.dram_tensor("m", [128, N_COLS], BF, kind="ExternalInput")
    xb_d = nc.dram_tensor("xb", [128, R_TILES], DT, kind="ExternalInput")
    out_d = nc.dram_tensor("out", [RPC, N_COLS], BF, kind="ExternalOutput")

    with tile.TileContext(nc) as tc:
        with (
            tc.tile_pool(name="inp", bufs=1) as sbin,
            tc.tile_pool(name="outp", bufs=4) as sbout,
            tc.tile_pool(name="ps", bufs=2, space="PSUM") as ps,
        ):
            # warm the ACT exp table-set load (~2.7 us) under the input DMAs
            warm = sbout.tile([128, 8], DT, name="actwarm")
            nc.scalar.activation(warm[:], warm[:], mybir.ActivationFunctionType.Exp)
            emit_body(nc, sbin, sbout, ps, q_d, m_d, xb_d, out_d,
                      tailsplit="all")
    finalize_nc(nc)
    return nc


def prepare_inputs(x, y):
    """Host-side prep: fp16/bf16 encodes, transposes, per-core maps."""
    x = np.asarray(x, dtype=np.float32)
    y = np.asarray(y, dtype=np.float32)
    assert x.shape == (N_ROWS, D) and y.shape == (N_COLS, D)

    x_sq = (x * x).sum(axis=1, dtype=np.float32)
    y_sq = (y * y).sum(axis=1, dtype=np.float32)

    x16 = x.astype(np.float16)
    yh = y.astype(ml_dtypes.bfloat16)
    y2h = (2.0 * yh.astype(np.float32)).astype(ml_dtypes.bfloat16)
    yl2 = (2.0 * (y - yh.astype(np.float32))).astype(ml_dtypes.bfloat16)
    ysq_h = y_sq.astype(ml_dtypes.bfloat16)
    ysq_l = (y_sq - ysq_h.astype(np.float32)).astype(ml_dtypes.bfloat16)

    # moving map, shared by all cores: [128, N_COLS] bf16
    m = np.zeros((128, N_COLS), ml_dtypes.bfloat16)
    m[:D] = y2h.T
    m[D] = -ysq_h
    m[D + 1] = -ysq_l
    m[D + 2:D + 2 + NL] = yl2.T[:NL]

    in_maps = []
    for c in range(N_CORES):
        rows = slice(c * RPC, (c + 1) * RPC)
        q = np.zeros((128, RPC), np.float16)
        q[:D] = x16.T[:, rows]
        q[D] = 1.0
        q[D + 1] = 1.0
        q[D + 2:D + 2 + NL] = x16.T[:NL, rows]
        xb = (-x_sq[rows]).astype(np.float32).reshape(R_TILES, 128).T.copy()
        in_maps.append({"q": q, "m": m, "xb": xb})
    return in_maps


def kernel(x, y):
    in_maps = prepare_inputs(x, y)
    nc = build_nc()
    res = run_bass_kernel_spmd(nc, in_maps, core_ids=list(range(N_CORES)))
    out = np.concatenate([res.results[c]["out"] for c in range(N_CORES)], axis=0)
    return out.astype(np.float32)
